# revision 11
# baseline (speedup 1.0000x reference)
"""GAT (3-layer graph attention + final linear) Trainium2 Bass kernel.

Problem: B=4 graphs, N=2048 atoms, D=128, H=256.
  per layer: h = relu(x @ W.T + b); e_ij = leaky_relu(f1_i + f2_j, 0.01)
  masked by adj; att = softmax_j(e); x = x + att @ h.
  final: relu(x @ Wt.T + bt).

Sharding: 8 cores; core c -> (graph b=c//2, row-half s=c%2 of the NxN
attention). Per-core the atom (j) axis is reordered to [own half |
other half] so the between-layer exchange can overlap compute: an
AllReduce(add) over the pair gives sum = mine + theirs, and
other = sum - mine (exact enough in fp32). Layer l's own-half work
(h, f, attention tiles j=0..7) proceeds while the collective runs.

Key structural points vs a naive port:
  - exp separability: exp(f1_i + f2_j) = exp(f1_i)*exp(f2_j). The NxN
    logits never exist: T1 = exp(f1) broadcast [128,NS] (PE K=1 outer
    product + ACT exp), per j-tile q1 = Copy(T1, scale=E2_j) on ACT,
    p = max(q1, T1s*e2_j) via one DVE scalar_tensor_tensor, then a
    0/1 bf16 adjacency mask multiply (DVE/GpSimd alternating). No PE
    mask-preload, no K=2 logit matmuls, no NxN exp on ACT.
  - leaky: exp(leaky(z)) = max(exp(z), exp(0.01 z)); no row-max needed
    (|z| bounded ~40, validated vs reference).
  - row sums without PE: S = (DVE free-axis reduce over the 16 p tiles)
    then (GpSimd partition reduce, axis=C) then a K=1 PE ones-outer
    broadcast + DVE reciprocal. Saves the 16K-column ones-matmul.
  - transposes off PE: adjacency (int32 -> 0/1 bf16) and h tiles are
    transposed with the DMA XBAR (dma_start(..., transpose=True)).
  - everything stays in transposed [feature, atom] layout; the residual
    and normalization are applied transposed, so per-layer natural-
    layout round trips are gone.

Hardware wait-slot discipline (walrus limits: DMA instr = 1 sem wait,
XPOSE DMA = 0, engine instr = ~2): excess waits are split onto
standalone EventSemaphore instructions by _legalize_waits.
"""

import numpy as np

import concourse.bass as bass
import concourse.mybir as mybir
import concourse.tile as tile
from concourse import masks
from concourse.bass_utils import run_bass_kernel_spmd

P = 128
F32 = mybir.dt.float32
BF16 = mybir.dt.bfloat16
I32 = mybir.dt.int32
AF = mybir.ActivationFunctionType
OP = mybir.AluOpType


def _legalize_waits(nc, dma_limit=1, engine_limit=1):
    """Walrus can encode only 1 sem wait on a DMA instruction, 0 on an
    XBAR-transpose DMA, and ~2 on an engine instruction. Move excess
    waits onto standalone EventSemaphore instructions (1 wait each)
    inserted just before the offender on the same engine."""
    counter = [0]

    def split(ins):
        si = ins.sync_info
        if si is None:
            return None
        tn = type(ins).__name__
        if tn == "InstDmaTransposeAnt":
            limit = 0
        elif tn.startswith("InstDMA"):
            limit = dma_limit
        else:
            limit = engine_limit
        waits = list(si.on_wait)
        if len(waits) <= limit:
            return None
        keep = waits[-limit:] if limit > 0 else []
        extra = waits[:-limit] if limit > 0 else waits
        evs = []
        for w in extra:
            counter[0] += 1
            evs.append(mybir.InstEventSemaphore(
                name=f"evsplit{counter[0]}", engine=ins.engine,
                sync_info=mybir.SyncInfo(on_wait=[w], on_update=[])))
        ins.sync_info = mybir.SyncInfo(on_wait=keep,
                                       on_update=list(si.on_update))
        return evs

    for f in nc.m.functions:
        for blk in f.blocks:
            new_list = []
            changed = False
            for ins in blk.instructions:
                evs = split(ins)
                if evs:
                    new_list.extend(evs)
                    changed = True
                new_list.append(ins)
            if changed:
                blk.instructions = new_list


def build_gat_nc(N, NS, D, H, num_cores, pair_groups, nlayers=3,
                 legalize=True):
    assert D == P and NS % 512 == 0 and N % 512 == 0
    nj = N // P        # j tiles (core-local atom order: 0..7 own, 8..15 other)
    njh = nj // 2
    nit = NS // P      # i tiles in shard
    nch = NS // 512    # 512-chunks in shard
    nchN = N // 512
    nH = H // P

    nc = bass.Bass("TRN2", target_bir_lowering=False, debug=False,
                   num_devices=num_cores)

    # ---- I/O ----
    xT_in = nc.dram_tensor("xT", [P, N], F32, kind="ExternalInput")
    adj_in = nc.dram_tensor("adj_s", [NS, N], I32, kind="ExternalInput")
    WT_in = [nc.dram_tensor(f"WT{l}", [D, D], F32, kind="ExternalInput")
             for l in range(nlayers)]
    bv_in = [nc.dram_tensor(f"bv{l}", [D, 1], F32, kind="ExternalInput")
             for l in range(nlayers)]
    av_in = [nc.dram_tensor(f"av{l}", [D, 2], F32, kind="ExternalInput")
             for l in range(nlayers)]
    WtT_in = nc.dram_tensor("WtT", [D, H], F32, kind="ExternalInput")
    btp_in = nc.dram_tensor("btp", [P, nH], F32, kind="ExternalInput")
    out_ext = nc.dram_tensor("out_s", [NS, H], F32, kind="ExternalOutput")

    # DRAM bounce buffers for the pair AllReduce of xTs shards
    ar_in = [nc.dram_tensor(f"ar_in{l}", [P, NS], F32)
             for l in range(nlayers - 1)]
    ar_out = [nc.dram_tensor(f"ar_out{l}", [P, NS], F32)
              for l in range(nlayers - 1)]

    with tile.TileContext(nc) as tc:
        import contextlib
        ctx = contextlib.ExitStack()
        with ctx:
            persist = ctx.enter_context(tc.tile_pool(name="persist", bufs=1))
            rawp = ctx.enter_context(tc.tile_pool(name="rawp", bufs=2))
            convp = ctx.enter_context(tc.tile_pool(name="convp", bufs=1))
            xtp = ctx.enter_context(tc.tile_pool(name="xtp", bufs=2))
            smallp = ctx.enter_context(tc.tile_pool(name="smallp", bufs=2))
            ocp = ctx.enter_context(tc.tile_pool(name="ocp", bufs=2))
            hp = ctx.enter_context(
                tc.tile_pool(name="hp", bufs=2, space="PSUM"))
            bigp = ctx.enter_context(
                tc.tile_pool(name="bigp", bufs=1, space="PSUM"))
            attp = ctx.enter_context(
                tc.tile_pool(name="attp", bufs=1, space="PSUM"))
            f2pp = ctx.enter_context(
                tc.tile_pool(name="f2pp", bufs=1, space="PSUM"))

            ident = persist.tile([P, P], F32)
            masks.make_identity(nc, ident[:])
            onesrow = persist.tile([1, P], F32)
            nc.vector.memset(onesrow[:], 1.0)

            # ---- persistent state ----
            adjT = [persist.tile([P, NS], BF16, name=f"adjT{j}",
                                 tag=f"adjT{j}") for j in range(nj)]
            p_all = persist.tile([P, nj, NS], BF16)
            hT = persist.tile([P, N], F32)
            hTb = persist.tile([P, N], BF16)
            hnat = [persist.tile([P, P], BF16, name=f"hnat{j}",
                                 tag=f"hnat{j}") for j in range(nj)]
            f1row = persist.tile([1, NS], F32)
            f2row = persist.tile([1, N], F32)
            f2c = persist.tile([P, nj], F32)
            E2c = persist.tile([P, nj], F32)
            e2c = persist.tile([P, nj], F32)
            T1 = persist.tile([P, NS], BF16)
            T1s = persist.tile([P, NS], BF16)
            srow = persist.tile([1, NS], F32)
            S1 = persist.tile([P, NS], F32)
            Rv = persist.tile([P, NS], F32)
            tmp = persist.tile([P, NS], F32)
            sumT = persist.tile([P, NS], F32)
            xTo = persist.tile([P, NS], F32)

            # raw DMA'd weights + DVE-laundered copies (so matmuls never
            # depend on two DMA queues)
            WT_d = [persist.tile([D, D], F32, name=f"WTd{l}", tag=f"WTd{l}")
                    for l in range(nlayers)]
            bv_d = [persist.tile([D, 1], F32, name=f"bvd{l}", tag=f"bvd{l}")
                    for l in range(nlayers)]
            av_d = [persist.tile([D, 2], F32, name=f"avd{l}", tag=f"avd{l}")
                    for l in range(nlayers)]
            WtT_d = persist.tile([D, H], F32)
            btp_d = persist.tile([P, nH], F32)
            WT = [persist.tile([D, D], F32, name=f"WTl{l}", tag=f"WTl{l}")
                  for l in range(nlayers)]
            bv = [persist.tile([D, 1], F32, name=f"bvl{l}", tag=f"bvl{l}")
                  for l in range(nlayers)]
            av = [persist.tile([D, 2], F32, name=f"avl{l}", tag=f"avl{l}")
                  for l in range(nlayers)]
            WtTt = persist.tile([D, H], F32)
            btpt = persist.tile([P, nH], F32)

            # ---- adjacency: DMA in (i32), convert to 0/1 bf16, DMA-XBAR
            # transpose into adjT[j][:, i] tiles. conv chunks all live at
            # once so the transposes can run j-major (adjT[0] done first).
            convs = []
            for it in range(nit):
                raw = rawp.tile([P, N], I32, name=f"raw{it}", tag="raw")
                nc.sync.dma_start(raw[:], adj_in.ap()[it * P:(it + 1) * P, :])
                conv = convp.tile([P, N], BF16, name=f"conv{it}",
                                  tag=f"conv{it}")
                nc.vector.tensor_copy(conv[:], raw[:])
                convs.append(conv)

            # ---- initial x / weights ----
            xT = persist.tile([P, N], F32)
            nc.sync.dma_start(xT[:], xT_in.ap())
            for l in range(nlayers):
                nc.sync.dma_start(WT_d[l][:], WT_in[l].ap())
                nc.sync.dma_start(bv_d[l][:], bv_in[l].ap())
                nc.sync.dma_start(av_d[l][:], av_in[l].ap())
                nc.vector.tensor_copy(WT[l][:], WT_d[l][:])
                nc.vector.tensor_copy(bv[l][:], bv_d[l][:])
                nc.vector.tensor_copy(av[l][:], av_d[l][:])
            nc.sync.dma_start(WtT_d[:], WtT_in.ap())
            nc.sync.dma_start(btp_d[:], btp_in.ap())
            nc.vector.tensor_copy(WtTt[:], WtT_d[:])
            nc.vector.tensor_copy(btpt[:], btp_d[:])

            # adjT transposes, j-major, alternating issue queues
            for j in range(nj):
                eng = nc.sync if j % 2 == 0 else nc.scalar
                for it in range(nit):
                    eng.dma_start(adjT[j][:, it * P:(it + 1) * P],
                                  convs[it][:, j * P:(j + 1) * P],
                                  transpose=True)

            # (tile, column offset) pair naming the current shard state;
            # layer 0 reads it straight out of the xT input tile.
            xts_t, xts_o = xT, 0

            def cur_xts(sl=None):
                if sl is None:
                    return xts_t[:, xts_o:xts_o + NS]
                return xts_t[:, xts_o + sl.start:xts_o + sl.stop]

            def do_half(l, half):
                """h, f, f2 cols, hnat for one half of the atom axis."""
                base = half * NS
                for ch in range(nch):
                    sl = slice(base + ch * 512, base + (ch + 1) * 512)
                    ssl = slice(ch * 512, (ch + 1) * 512)
                    if l == 0:
                        src = xT[:, sl]
                    elif half == 0:
                        src = cur_xts(ssl)
                    else:
                        src = xTo[:, ssl]
                    ps = hp.tile([P, 512], F32, name=f"hps{l}_{half}_{ch}",
                                 tag="hps")
                    nc.tensor.matmul(ps[:], WT[l][:], src,
                                     start=True, stop=True)
                    nc.vector.tensor_scalar(hT[:, sl], ps[:], bv[l][:], 0.0,
                                            OP.add, OP.max)
                    nc.scalar.activation(hTb[:, sl], ps[:], AF.Relu,
                                         bias=bv[l][:])
                for ch in range(nch):
                    sl = slice(base + ch * 512, base + (ch + 1) * 512)
                    psf = f2pp.tile([1, 512], F32,
                                    name=f"fps{l}_{half}_{ch}", tag="fps")
                    nc.tensor.matmul(psf[:], av[l][:, 1:2], hT[:, sl],
                                     start=True, stop=True)
                    nc.vector.tensor_copy(f2row[:, sl], psf[:])
                    if half == 0:
                        psg = f2pp.tile([1, 512], F32,
                                        name=f"gps{l}_{ch}", tag="fps")
                        nc.tensor.matmul(psg[:], av[l][:, 0:1], hT[:, sl],
                                         start=True, stop=True)
                        nc.vector.tensor_copy(
                            f1row[:, ch * 512:(ch + 1) * 512], psg[:])
                # f2 row -> per-partition columns via PE 1-col transposes
                psf2 = f2pp.tile([P, njh], F32, name=f"f2ps{l}_{half}",
                                 tag="f2")
                for q in range(njh):
                    j = half * njh + q
                    nc.tensor.transpose(psf2[:, q:q + 1],
                                        f2row[0:1, j * P:(j + 1) * P],
                                        ident[0:1, 0:1])
                jsl = slice(half * njh, (half + 1) * njh)
                nc.vector.tensor_copy(f2c[:, jsl], psf2[:])
                nc.scalar.activation(E2c[:, jsl], f2c[:, jsl], AF.Exp)
                nc.scalar.activation(e2c[:, jsl], f2c[:, jsl], AF.Exp,
                                     scale=0.01)
                # h natural tiles via DMA XBAR transpose
                for q in range(njh):
                    j = half * njh + q
                    nc.sync.dma_start(hnat[j][:],
                                      hTb[:, j * P:(j + 1) * P],
                                      transpose=True)

            def do_jtiles(l, psAT, j0, j1):
                for j in range(j0, j1):
                    pj = p_all[:, j, :]
                    nc.scalar.activation(pj, T1[:], AF.Copy,
                                         scale=E2c[:, j:j + 1])
                    nc.vector.scalar_tensor_tensor(pj, T1s[:],
                                                   e2c[:, j:j + 1], pj,
                                                   OP.mult, OP.max)
                    if j % 2 == 0:
                        nc.gpsimd.tensor_tensor(pj, pj, adjT[j][:], OP.mult)
                    else:
                        nc.vector.tensor_tensor(pj, pj, adjT[j][:], OP.mult)
                    for ch in range(nch):
                        sl = slice(ch * 512, (ch + 1) * 512)
                        nc.tensor.matmul(psAT[:, sl], hnat[j][:],
                                         p_all[:, j, sl],
                                         start=(j == 0), stop=(j == nj - 1))

            for l in range(nlayers):
                last = l == nlayers - 1
                # own half (no collective dependency)
                do_half(l, 0)
                # T1 / T1s from f1 own rows: PE ones-outer bcast + ACT exp
                psF1 = bigp.tile([P, NS], F32, name=f"f1b{l}", tag="big")
                for ch in range(nch):
                    sl = slice(ch * 512, (ch + 1) * 512)
                    nc.tensor.matmul(psF1[:, sl], onesrow[:],
                                     f1row[0:1, sl], start=True, stop=True)
                nc.scalar.activation(T1[:], psF1[:], AF.Exp)
                nc.scalar.activation(T1s[:], psF1[:], AF.Exp, scale=0.01)

                psAT = attp.tile([P, NS], F32, name=f"psAT{l}", tag="att")
                do_jtiles(l, psAT, 0, njh)

                # other half: needs peer's shard (layer 0: from xT input)
                if l > 0:
                    nc.sync.dma_start(sumT[:], ar_out[l - 1].ap())
                    nc.vector.tensor_tensor(xTo[:], sumT[:], cur_xts(),
                                            OP.subtract)
                do_half(l, 1)
                do_jtiles(l, psAT, njh, nj)

                # S = sum_j p: DVE reduce over tiles, gpsimd partition
                # reduce, PE ones-outer broadcast, DVE reciprocal.
                nc.vector.tensor_reduce(
                    S1[:], p_all[:].rearrange("p a b -> p b a"),
                    mybir.AxisListType.X, OP.add)
                nc.gpsimd.tensor_reduce(srow[:], S1[:],
                                        mybir.AxisListType.C, OP.add)
                psR = bigp.tile([P, NS], F32, name=f"rb{l}", tag="big")
                for ch in range(nch):
                    sl = slice(ch * 512, (ch + 1) * 512)
                    nc.tensor.matmul(psR[:, sl], onesrow[:], srow[:, sl],
                                     start=True, stop=True)
                nc.vector.reciprocal(Rv[:], psR[:])

                # xTs_new = psAT * R + xTs_cur  (all transposed layout)
                nc.vector.tensor_tensor(tmp[:], psAT[:], Rv[:], OP.mult)
                xTs_new = xtp.tile([P, NS], F32, name=f"xTs{l + 1}",
                                   tag="xTs")
                nc.vector.tensor_tensor(xTs_new[:], tmp[:], cur_xts(),
                                        OP.add)
                xts_t, xts_o = xTs_new, 0

                if not last:
                    nc.gpsimd.dma_start(ar_in[l].ap(), xTs_new[:])
                    nc.gpsimd.collective_compute(
                        "AllReduce", OP.add, replica_groups=pair_groups,
                        ins=[ar_in[l].ap()], outs=[ar_out[l].ap()])

            # ---- final linear: out = relu(x @ Wt.T + bt) ----
            for g in range(nH):
                for ch in range(nch):
                    sl = slice(ch * 512, (ch + 1) * 512)
                    ps = hp.tile([P, 512], F32, name=f"ops{g}_{ch}",
                                 tag="hps")
                    nc.tensor.matmul(ps[:], WtTt[:, g * P:(g + 1) * P],
                                     cur_xts(sl), start=True, stop=True)
                    oc = ocp.tile([P, 512], F32, name=f"oc{g}_{ch}",
                                  tag="oc")
                    nc.vector.tensor_scalar(oc[:], ps[:], btpt[:, g:g + 1],
                                            0.0, OP.add, OP.max)
                    pst = hp.tile([P, 512], F32, name=f"otp{g}_{ch}",
                                  tag="hps")
                    for q in range(4):
                        nc.tensor.transpose(pst[:, q * P:(q + 1) * P],
                                            oc[:, q * P:(q + 1) * P],
                                            ident[:])
                    for q in range(4):
                        k = ch * 4 + q
                        ob = smallp.tile([P, P], F32, name=f"ob{g}_{k}",
                                         tag="ob")
                        nc.vector.tensor_copy(ob[:],
                                              pst[:, q * P:(q + 1) * P])
                        nc.sync.dma_start(
                            out_ext.ap()[k * P:(k + 1) * P,
                                         g * P:(g + 1) * P],
                            ob[:])

    if legalize:
        _legalize_waits(nc)
    return nc


def make_in_maps(x, adj, Ws, bs, avs, Wt, bt, num_cores, NS):
    """Per-core input dicts. Core c -> (graph c//2, row-half c%2).
    Per-core the atom (column) axis is permuted to [own half | other]."""
    B, N, D = x.shape
    H = Wt.shape[0]
    nH = H // P
    x = np.ascontiguousarray(x, np.float32)
    adj = np.ascontiguousarray(adj, np.int32)
    shared = {"WtT": np.ascontiguousarray(np.asarray(Wt, np.float32).T),
              "btp": np.ascontiguousarray(
                  np.asarray(bt, np.float32).reshape(nH, P).T)}
    for l, (W, b, a) in enumerate(zip(Ws, bs, avs)):
        shared[f"WT{l}"] = np.ascontiguousarray(np.asarray(W, np.float32).T)
        shared[f"bv{l}"] = np.ascontiguousarray(
            np.asarray(b, np.float32).reshape(D, 1))
        shared[f"av{l}"] = np.ascontiguousarray(
            np.stack([np.asarray(a, np.float32)[:D, 0],
                      np.asarray(a, np.float32)[D:, 0]], axis=1))
    in_maps = []
    for c in range(num_cores):
        b, s = c // 2, c % 2
        own = slice(s * NS, (s + 1) * NS)
        oth = slice((1 - s) * NS, (2 - s) * NS)
        perm = np.concatenate([np.arange(s * NS, (s + 1) * NS),
                               np.arange((1 - s) * NS, (2 - s) * NS)])
        m = dict(shared)
        m["xT"] = np.ascontiguousarray(x[b][perm].T)
        m["adj_s"] = np.ascontiguousarray(adj[b, s * NS:(s + 1) * NS][:, perm])
        in_maps.append(m)
    return in_maps


_NC_CACHE = {}


def kernel(x, adj, W0, b0, W1, b1, W2, b2, a0, a1, a2, Wt, bt):
    B, N, D = 4, 2048, 128
    H = 256
    NUM_CORES = 8
    NS = N // 2
    pair_groups = [[2 * i, 2 * i + 1] for i in range(NUM_CORES // 2)]

    key = (N, NS, D, H, NUM_CORES)
    if key not in _NC_CACHE:
        _NC_CACHE[key] = build_gat_nc(N, NS, D, H, NUM_CORES, pair_groups)
    nc = _NC_CACHE[key]

    in_maps = make_in_maps(np.asarray(x), np.asarray(adj),
                           [W0, W1, W2], [b0, b1, b2], [a0, a1, a2],
                           np.asarray(Wt), np.asarray(bt), NUM_CORES, NS)
    res = run_bass_kernel_spmd(nc, in_maps, list(range(NUM_CORES))).results
    out = np.empty((B, N, H), np.float32)
    for c in range(NUM_CORES):
        b, s = c // 2, c % 2
        out[b, s * NS:(s + 1) * NS, :] = res[c]["out_s"]
    return out


# revision 14
# speedup vs baseline: 3.4654x; 3.4654x over previous
"""GAT (3-layer graph attention + final linear) Trainium2 Bass kernel.

Problem: B=4 graphs, N=2048 atoms, D=128, H=256.
  per layer: h = relu(x @ W.T + b); e_ij = leaky_relu(f1_i + f2_j, 0.01)
  masked by adj; att = softmax_j(e); x = x + att @ h.
  final: relu(x @ Wt.T + bt).

Sharding: 8 cores; core c -> (graph b=c//2, row-half s=c%2 of the NxN
attention). Per-core the atom (j) axis is reordered to [own half |
other half] so the between-layer exchange can overlap compute: an
AllReduce(add) over the pair gives sum = mine + theirs, and
other = sum - mine (exact enough in fp32). Layer l's own-half work
(h, f, attention tiles j=0..7) proceeds while the collective runs.

Key structural points vs a naive port:
  - exp separability: exp(f1_i + f2_j) = exp(f1_i)*exp(f2_j). The NxN
    logits never exist: T1 = exp(f1) broadcast [128,NS] (PE K=1 outer
    product + ACT exp), per j-tile q1 = Copy(T1, scale=E2_j) on ACT,
    p = max(q1, T1s*e2_j) via one DVE scalar_tensor_tensor, then a
    0/1 bf16 adjacency mask multiply (DVE/GpSimd alternating). No PE
    mask-preload, no K=2 logit matmuls, no NxN exp on ACT.
  - leaky: exp(leaky(z)) = max(exp(z), exp(0.01 z)); no row-max needed
    (|z| bounded ~40, validated vs reference).
  - row sums via a ones-column PE matmul accumulated alongside psAT;
    1/S via K=1 PE ones-outer broadcast + DVE reciprocal.
  - adjacency (int32 -> 0/1 bf16) and h tiles transposed on PE (the
    DMA XBAR's per-instruction issue cost of ~1.2us is too high).
  - everything stays in transposed [feature, atom] layout; the residual
    and normalization are applied transposed, so per-layer natural-
    layout round trips are gone.

Hardware wait-slot discipline (walrus limits: DMA instr = 1 sem wait,
XPOSE DMA = 0, engine instr = ~2): excess waits are split onto
standalone EventSemaphore instructions by _legalize_waits.
"""

import numpy as np

import concourse.bass as bass
import concourse.mybir as mybir
import concourse.tile as tile
from concourse import masks
from concourse.bass_utils import run_bass_kernel_spmd

P = 128
F32 = mybir.dt.float32
BF16 = mybir.dt.bfloat16
I32 = mybir.dt.int32
AF = mybir.ActivationFunctionType
OP = mybir.AluOpType


def _legalize_waits(nc, dma_limit=1, engine_limit=1):
    """Walrus can encode only 1 sem wait on a DMA instruction, 0 on an
    XBAR-transpose DMA, and ~2 on an engine instruction. Move excess
    waits onto standalone EventSemaphore instructions (1 wait each)
    inserted just before the offender on the same engine."""
    counter = [0]

    def split(ins):
        si = ins.sync_info
        if si is None:
            return None
        tn = type(ins).__name__
        if tn == "InstDmaTransposeAnt":
            limit = 0
        elif tn.startswith("InstDMA"):
            limit = dma_limit
        else:
            limit = engine_limit
        waits = list(si.on_wait)
        if len(waits) <= limit:
            return None
        keep = waits[-limit:] if limit > 0 else []
        extra = waits[:-limit] if limit > 0 else waits
        evs = []
        for w in extra:
            counter[0] += 1
            evs.append(mybir.InstEventSemaphore(
                name=f"evsplit{counter[0]}", engine=ins.engine,
                sync_info=mybir.SyncInfo(on_wait=[w], on_update=[])))
        ins.sync_info = mybir.SyncInfo(on_wait=keep,
                                       on_update=list(si.on_update))
        return evs

    for f in nc.m.functions:
        for blk in f.blocks:
            new_list = []
            changed = False
            for ins in blk.instructions:
                evs = split(ins)
                if evs:
                    new_list.extend(evs)
                    changed = True
                new_list.append(ins)
            if changed:
                blk.instructions = new_list


def build_gat_nc(N, NS, D, H, num_cores, pair_groups, nlayers=3,
                 legalize=True):
    assert D == P and NS % 512 == 0 and N % 512 == 0
    nj = N // P        # j tiles (core-local atom order: 0..7 own, 8..15 other)
    njh = nj // 2
    nit = NS // P      # i tiles in shard
    nch = NS // 512    # 512-chunks in shard
    nchN = N // 512
    nH = H // P

    nc = bass.Bass("TRN2", target_bir_lowering=False, debug=False,
                   num_devices=num_cores)

    # ---- I/O ----
    xT_in = nc.dram_tensor("xT", [P, N], F32, kind="ExternalInput")
    adj_in = nc.dram_tensor("adj_s", [NS, N], I32, kind="ExternalInput")
    WT_in = [nc.dram_tensor(f"WT{l}", [D, D], F32, kind="ExternalInput")
             for l in range(nlayers)]
    bv_in = [nc.dram_tensor(f"bv{l}", [D, 1], F32, kind="ExternalInput")
             for l in range(nlayers)]
    av_in = [nc.dram_tensor(f"av{l}", [D, 2], F32, kind="ExternalInput")
             for l in range(nlayers)]
    WtT_in = nc.dram_tensor("WtT", [D, H], F32, kind="ExternalInput")
    btp_in = nc.dram_tensor("btp", [P, nH], F32, kind="ExternalInput")
    out_ext = nc.dram_tensor("out_s", [NS, H], F32, kind="ExternalOutput")

    # DRAM bounce buffers for the pair AllReduce of xTs shards
    ar_in = [nc.dram_tensor(f"ar_in{l}", [P, NS], F32)
             for l in range(nlayers - 1)]
    ar_out = [nc.dram_tensor(f"ar_out{l}", [P, NS], F32)
              for l in range(nlayers - 1)]

    with tile.TileContext(nc) as tc:
        import contextlib
        ctx = contextlib.ExitStack()
        with ctx:
            persist = ctx.enter_context(tc.tile_pool(name="persist", bufs=1))
            rawp = ctx.enter_context(tc.tile_pool(name="rawp", bufs=2))
            convp = ctx.enter_context(tc.tile_pool(name="convp", bufs=1))
            xtp = ctx.enter_context(tc.tile_pool(name="xtp", bufs=2))
            qp = ctx.enter_context(tc.tile_pool(name="qp", bufs=2))
            smallp = ctx.enter_context(tc.tile_pool(name="smallp", bufs=2))
            ocp = ctx.enter_context(tc.tile_pool(name="ocp", bufs=2))
            hp = ctx.enter_context(
                tc.tile_pool(name="hp", bufs=2, space="PSUM"))
            bigp = ctx.enter_context(
                tc.tile_pool(name="bigp", bufs=1, space="PSUM"))
            attp = ctx.enter_context(
                tc.tile_pool(name="attp", bufs=1, space="PSUM"))
            f2pp = ctx.enter_context(
                tc.tile_pool(name="f2pp", bufs=1, space="PSUM"))

            ident = persist.tile([P, P], F32)
            masks.make_identity(nc, ident[:])
            onesrow = persist.tile([1, P], F32)
            nc.vector.memset(onesrow[:], 1.0)
            onescol = persist.tile([P, 1], BF16)
            nc.vector.memset(onescol[:], 1.0)
            identb = persist.tile([P, P], BF16)
            masks.make_identity(nc, identb[:])

            # ---- persistent state ----
            adjT = [persist.tile([P, NS], BF16, name=f"adjT{j}",
                                 tag=f"adjT{j}") for j in range(nj)]
            p_all = persist.tile([P, nj, NS], BF16)
            hT = persist.tile([P, N], F32)
            hnat = [persist.tile([P, P], BF16, name=f"hnat{j}",
                                 tag=f"hnat{j}") for j in range(nj)]
            f1row = persist.tile([1, NS], F32)
            f2row = persist.tile([1, N], F32)
            f2c = persist.tile([P, nj], F32)
            E2c = persist.tile([P, nj], F32)
            e2c = persist.tile([P, nj], F32)
            T1 = persist.tile([P, NS], BF16)
            T1s = persist.tile([P, NS], BF16)
            srow = persist.tile([1, NS], F32)
            Rv = persist.tile([P, NS], F32)
            tmp = persist.tile([P, NS], F32)
            sumT = persist.tile([P, NS], F32)
            xTo = persist.tile([P, NS], F32)

            # raw DMA'd weights + DVE-laundered copies (so matmuls never
            # depend on two DMA queues)
            WT_d = [persist.tile([D, D], F32, name=f"WTd{l}", tag=f"WTd{l}")
                    for l in range(nlayers)]
            bv_d = [persist.tile([D, 1], F32, name=f"bvd{l}", tag=f"bvd{l}")
                    for l in range(nlayers)]
            av_d = [persist.tile([D, 2], F32, name=f"avd{l}", tag=f"avd{l}")
                    for l in range(nlayers)]
            WtT_d = persist.tile([D, H], F32)
            btp_d = persist.tile([P, nH], F32)
            WT = [persist.tile([D, D], F32, name=f"WTl{l}", tag=f"WTl{l}")
                  for l in range(nlayers)]
            bv = [persist.tile([D, 1], F32, name=f"bvl{l}", tag=f"bvl{l}")
                  for l in range(nlayers)]
            av = [persist.tile([D, 2], F32, name=f"avl{l}", tag=f"avl{l}")
                  for l in range(nlayers)]
            WtTt = persist.tile([D, H], F32)
            btpt = persist.tile([P, nH], F32)

            # ---- adjacency: DMA in (i32), convert to 0/1 bf16 on DVE,
            # transpose on PE (grouped 4-wide through PSUM).
            convs = []
            for it in range(nit):
                raw = rawp.tile([P, N], I32, name=f"raw{it}", tag="raw")
                nc.sync.dma_start(raw[:], adj_in.ap()[it * P:(it + 1) * P, :])
                conv = convp.tile([P, N], BF16, name=f"conv{it}",
                                  tag=f"conv{it}")
                nc.vector.tensor_copy(conv[:], raw[:])
                convs.append(conv)

            # ---- initial x / weights ----
            xT = persist.tile([P, N], F32)
            nc.sync.dma_start(xT[:], xT_in.ap())
            for l in range(nlayers):
                nc.sync.dma_start(WT_d[l][:], WT_in[l].ap())
                nc.sync.dma_start(bv_d[l][:], bv_in[l].ap())
                nc.sync.dma_start(av_d[l][:], av_in[l].ap())
                nc.vector.tensor_copy(WT[l][:], WT_d[l][:])
                nc.vector.tensor_copy(bv[l][:], bv_d[l][:])
                nc.vector.tensor_copy(av[l][:], av_d[l][:])
            nc.sync.dma_start(WtT_d[:], WtT_in.ap())
            nc.sync.dma_start(btp_d[:], btp_in.ap())
            nc.vector.tensor_copy(WtTt[:], WtT_d[:])
            nc.vector.tensor_copy(btpt[:], btp_d[:])

            def prep_adjT():
                for j in range(nj):
                    for itg in range(nit // 4):
                        pst = hp.tile([P, 512], BF16,
                                      name=f"tp{itg}_{j}", tag="hps")
                        for q in range(4):
                            it = itg * 4 + q
                            nc.tensor.transpose(
                                pst[:, q * P:(q + 1) * P],
                                convs[it][:, j * P:(j + 1) * P], identb[:])
                        nc.vector.tensor_copy(
                            adjT[j][:, itg * 512:(itg + 1) * 512], pst[:])

            # (tile, column offset) pair naming the current shard state;
            # layer 0 reads it straight out of the xT input tile.
            xts_t, xts_o = xT, 0

            def cur_xts(sl=None):
                if sl is None:
                    return xts_t[:, xts_o:xts_o + NS]
                return xts_t[:, xts_o + sl.start:xts_o + sl.stop]

            def do_half(l, half):
                """h, f, f2 cols, hnat for one half of the atom axis."""
                base = half * NS
                for ch in range(nch):
                    sl = slice(base + ch * 512, base + (ch + 1) * 512)
                    ssl = slice(ch * 512, (ch + 1) * 512)
                    if l == 0:
                        src = xT[:, sl]
                    elif half == 0:
                        src = cur_xts(ssl)
                    else:
                        src = xTo[:, ssl]
                    ps = hp.tile([P, 512], F32, name=f"hps{l}_{half}_{ch}",
                                 tag="hps")
                    nc.tensor.matmul(ps[:], WT[l][:], src,
                                     start=True, stop=True)
                    nc.vector.tensor_scalar(hT[:, sl], ps[:], bv[l][:], 0.0,
                                            OP.add, OP.max)
                for ch in range(nch):
                    sl = slice(base + ch * 512, base + (ch + 1) * 512)
                    psf = f2pp.tile([1, 512], F32,
                                    name=f"fps{l}_{half}_{ch}", tag="fps")
                    nc.tensor.matmul(psf[:], av[l][:, 1:2], hT[:, sl],
                                     start=True, stop=True)
                    nc.vector.tensor_copy(f2row[:, sl], psf[:])
                    if half == 0:
                        psg = f2pp.tile([1, 512], F32,
                                        name=f"gps{l}_{ch}", tag="fps")
                        nc.tensor.matmul(psg[:], av[l][:, 0:1], hT[:, sl],
                                         start=True, stop=True)
                        nc.vector.tensor_copy(
                            f1row[:, ch * 512:(ch + 1) * 512], psg[:])
                # f2 row -> per-partition columns via PE 1-col transposes
                psf2 = f2pp.tile([P, njh], F32, name=f"f2ps{l}_{half}",
                                 tag="f2")
                for q in range(njh):
                    j = half * njh + q
                    nc.tensor.transpose(psf2[:, q:q + 1],
                                        f2row[0:1, j * P:(j + 1) * P],
                                        ident[0:1, 0:1])
                jsl = slice(half * njh, (half + 1) * njh)
                nc.vector.tensor_copy(f2c[:, jsl], psf2[:])
                nc.scalar.activation(E2c[:, jsl], f2c[:, jsl], AF.Exp)
                nc.scalar.activation(e2c[:, jsl], f2c[:, jsl], AF.Exp,
                                     scale=0.01)
                # h natural tiles via PE transposes (4-wide groups)
                for g in range(njh // 4):
                    pst = hp.tile([P, 512], F32,
                                  name=f"htp{l}_{half}_{g}", tag="hps")
                    for q in range(4):
                        j = half * njh + g * 4 + q
                        nc.tensor.transpose(pst[:, q * P:(q + 1) * P],
                                            hT[:, j * P:(j + 1) * P],
                                            ident[:])
                    for q in range(4):
                        j = half * njh + g * 4 + q
                        nc.vector.tensor_copy(hnat[j][:],
                                              pst[:, q * P:(q + 1) * P])

            def do_jtiles(l, psAT, psS, j0, j1):
                for j in range(j0, j1):
                    pj = p_all[:, j, :]
                    q2 = qp.tile([P, NS], BF16, name=f"q2_{l}_{j}",
                                 tag="q2", bufs=2)
                    nc.scalar.activation(pj, T1[:], AF.Copy,
                                         scale=E2c[:, j:j + 1])
                    nc.scalar.activation(q2[:], T1s[:], AF.Copy,
                                         scale=e2c[:, j:j + 1])
                    nc.vector.tensor_tensor(pj, pj, q2[:], OP.max)
                    nc.vector.tensor_tensor(pj, pj, adjT[j][:], OP.mult)
                    for ch in range(nch):
                        sl = slice(ch * 512, (ch + 1) * 512)
                        nc.tensor.matmul(psAT[:, sl], hnat[j][:],
                                         p_all[:, j, sl],
                                         start=(j == 0), stop=(j == nj - 1))
                        nc.tensor.matmul(psS[0:1, sl], onescol[:],
                                         p_all[:, j, sl],
                                         start=(j == 0), stop=(j == nj - 1))

            for l in range(nlayers):
                last = l == nlayers - 1
                # own half (no collective dependency)
                do_half(l, 0)
                # T1 / T1s from f1 own rows: PE ones-outer bcast + ACT exp
                psF1 = bigp.tile([P, NS], F32, name=f"f1b{l}", tag="big")
                for ch in range(nch):
                    sl = slice(ch * 512, (ch + 1) * 512)
                    nc.tensor.matmul(psF1[:, sl], onesrow[:],
                                     f1row[0:1, sl], start=True, stop=True)
                nc.scalar.activation(T1[:], psF1[:], AF.Exp)
                nc.scalar.activation(T1s[:], psF1[:], AF.Exp, scale=0.01)
                if l == 0:
                    prep_adjT()

                psAT = attp.tile([P, NS], F32, name=f"psAT{l}", tag="att")
                psS = bigp.tile([1, NS], F32, name=f"psS{l}", tag="big")
                do_jtiles(l, psAT, psS, 0, njh)

                # other half: needs peer's shard (layer 0: from xT input)
                if l > 0:
                    nc.sync.dma_start(sumT[:], ar_out[l - 1].ap())
                    nc.vector.tensor_tensor(xTo[:], sumT[:], cur_xts(),
                                            OP.subtract)
                do_half(l, 1)
                do_jtiles(l, psAT, psS, njh, nj)

                # 1/S: copy S row to SBUF, PE ones-outer broadcast,
                # fast approximate reciprocal.
                nc.vector.tensor_copy(srow[:], psS[:])
                psR = bigp.tile([P, NS], F32, name=f"rb{l}", tag="big")
                for ch in range(nch):
                    sl = slice(ch * 512, (ch + 1) * 512)
                    nc.tensor.matmul(psR[:, sl], onesrow[:], srow[0:1, sl],
                                     start=True, stop=True)
                nc.vector.reciprocal(Rv[:], psR[:])

                # xTs_new = psAT * R + xTs_cur  (all transposed layout)
                nc.vector.tensor_tensor(tmp[:], psAT[:], Rv[:], OP.mult)
                xTs_new = xtp.tile([P, NS], F32, name=f"xTs{l + 1}",
                                   tag="xTs")
                nc.vector.tensor_tensor(xTs_new[:], tmp[:], cur_xts(),
                                        OP.add)
                xts_t, xts_o = xTs_new, 0

                if not last:
                    nc.gpsimd.dma_start(ar_in[l].ap(), xTs_new[:])
                    nc.gpsimd.collective_compute(
                        "AllReduce", OP.add, replica_groups=pair_groups,
                        ins=[ar_in[l].ap()], outs=[ar_out[l].ap()])

            # ---- final linear: out = relu(x @ Wt.T + bt) ----
            for g in range(nH):
                for ch in range(nch):
                    sl = slice(ch * 512, (ch + 1) * 512)
                    ps = hp.tile([P, 512], F32, name=f"ops{g}_{ch}",
                                 tag="hps")
                    nc.tensor.matmul(ps[:], WtTt[:, g * P:(g + 1) * P],
                                     cur_xts(sl), start=True, stop=True)
                    oc = ocp.tile([P, 512], F32, name=f"oc{g}_{ch}",
                                  tag="oc")
                    nc.vector.tensor_scalar(oc[:], ps[:], btpt[:, g:g + 1],
                                            0.0, OP.add, OP.max)
                    pst = hp.tile([P, 512], F32, name=f"otp{g}_{ch}",
                                  tag="hps")
                    for q in range(4):
                        nc.tensor.transpose(pst[:, q * P:(q + 1) * P],
                                            oc[:, q * P:(q + 1) * P],
                                            ident[:])
                    for q in range(4):
                        k = ch * 4 + q
                        ob = smallp.tile([P, P], F32, name=f"ob{g}_{k}",
                                         tag="ob")
                        nc.vector.tensor_copy(ob[:],
                                              pst[:, q * P:(q + 1) * P])
                        nc.sync.dma_start(
                            out_ext.ap()[k * P:(k + 1) * P,
                                         g * P:(g + 1) * P],
                            ob[:])

    if legalize:
        _legalize_waits(nc)
    return nc


def make_in_maps(x, adj, Ws, bs, avs, Wt, bt, num_cores, NS):
    """Per-core input dicts. Core c -> (graph c//2, row-half c%2).
    Per-core the atom (column) axis is permuted to [own half | other]."""
    B, N, D = x.shape
    H = Wt.shape[0]
    nH = H // P
    x = np.ascontiguousarray(x, np.float32)
    adj = np.ascontiguousarray(adj, np.int32)
    shared = {"WtT": np.ascontiguousarray(np.asarray(Wt, np.float32).T),
              "btp": np.ascontiguousarray(
                  np.asarray(bt, np.float32).reshape(nH, P).T)}
    for l, (W, b, a) in enumerate(zip(Ws, bs, avs)):
        shared[f"WT{l}"] = np.ascontiguousarray(np.asarray(W, np.float32).T)
        shared[f"bv{l}"] = np.ascontiguousarray(
            np.asarray(b, np.float32).reshape(D, 1))
        shared[f"av{l}"] = np.ascontiguousarray(
            np.stack([np.asarray(a, np.float32)[:D, 0],
                      np.asarray(a, np.float32)[D:, 0]], axis=1))
    in_maps = []
    for c in range(num_cores):
        b, s = c // 2, c % 2
        own = slice(s * NS, (s + 1) * NS)
        oth = slice((1 - s) * NS, (2 - s) * NS)
        perm = np.concatenate([np.arange(s * NS, (s + 1) * NS),
                               np.arange((1 - s) * NS, (2 - s) * NS)])
        m = dict(shared)
        m["xT"] = np.ascontiguousarray(x[b][perm].T)
        m["adj_s"] = np.ascontiguousarray(adj[b, s * NS:(s + 1) * NS][:, perm])
        in_maps.append(m)
    return in_maps


_NC_CACHE = {}


def kernel(x, adj, W0, b0, W1, b1, W2, b2, a0, a1, a2, Wt, bt):
    B, N, D = 4, 2048, 128
    H = 256
    NUM_CORES = 8
    NS = N // 2
    pair_groups = [[2 * i, 2 * i + 1] for i in range(NUM_CORES // 2)]

    key = (N, NS, D, H, NUM_CORES)
    if key not in _NC_CACHE:
        _NC_CACHE[key] = build_gat_nc(N, NS, D, H, NUM_CORES, pair_groups)
    nc = _NC_CACHE[key]

    in_maps = make_in_maps(np.asarray(x), np.asarray(adj),
                           [W0, W1, W2], [b0, b1, b2], [a0, a1, a2],
                           np.asarray(Wt), np.asarray(bt), NUM_CORES, NS)
    res = run_bass_kernel_spmd(nc, in_maps, list(range(NUM_CORES))).results
    out = np.empty((B, N, H), np.float32)
    for c in range(NUM_CORES):
        b, s = c // 2, c % 2
        out[b, s * NS:(s + 1) * NS, :] = res[c]["out_s"]
    return out
